# revision 15
# baseline (speedup 1.0000x reference)
"""Trainium2 Bass kernel for a 2-layer GAT node classifier (SPMD over 8 NeuronCores).

Strategy (per layer):
  - Replicated dense phase: every core computes the full projection table
    H'[n] = [x @ W | x @ (W B_l)] (h plus the per-head left-attention dot),
    written to per-core HBM gather tables. The right-attention dot er is kept
    only for the core's own destination-node range, resident in SBUF.
  - Edge phase: destination nodes are sharded contiguously across cores
    (6250 per core). Per core, edges sorted by destination, grouped into
    128-node destination blocks, padded to 128-edge tiles, and split into
    "lo"/"hi" source groups so the int16 gather indices can address the
    whole 50048-row table via two base tables.
  - Per 1024-edge gather group (one SWDGE dma_gather instruction): build
    selection matrices SEL (edges x nodes one-hot by destination) and its
    transpose via iota-compare + a K=1 PE broadcast matmul, compute
    unnormalized attention weights w = exp(leaky_relu(el[src] + er[dst])),
    and accumulate [sum w*h[src] | sum w] per destination block with PE
    matmuls (exact segment-sum via one-hot matmul). The edge softmax
    denominator is applied per node at the end (alpha never materialized;
    segment-max is skipped — scores are O(1) so exp is safe).
  - Epilogue per block: divide by denominator, add bias, (layer 1: ELU),
    write the core's output slice.

The host only does index preprocessing (graph partition / sort / padding),
weight repacking (folding attention vectors into the weight matrix:
W @ blockdiag(attn)), transposes of inputs, and concatenation of outputs.
All floating-point compute on the 800k edges / 50k nodes runs on device.
"""

import math
import numpy as np
import ml_dtypes

BF16_NP = ml_dtypes.bfloat16

import concourse.bacc as bacc
import concourse.tile as tile
from concourse.tile_rust import add_dep_helper
from concourse import mybir
from concourse.bass_utils import run_bass_kernel_spmd

P = 128
N_CORES = 8
AluOp = mybir.AluOpType
ActFn = mybir.ActivationFunctionType
F32 = mybir.dt.float32
BF16 = mybir.dt.bfloat16
I16 = mybir.dt.int16

# Problem constants (nn_GAT_Node_Classifier)
N_NODES = 50000
N_EDGES = 800000
IN_DIM = 256
HID = 32
HEADS = 8
OUT_DIM = 16
NEG_SLOPE = 0.2


class LayerCfg:
    def __init__(self, n_in, n_h, heads, elem, n_nodes, n_cores, split, elu):
        self.n_in = n_in              # input feature dim (must be mult of 128)
        self.n_h = n_h                # heads * hid
        self.heads = heads
        self.hid = n_h // heads
        self.elem = elem              # gather row floats (>= n_h + heads, 64B mult)
        self.n_cores = n_cores
        self.nodes_per_core = n_nodes // n_cores
        self.blocks = (self.nodes_per_core + P - 1) // P
        self.npad = self.blocks * P
        self.n_nodes = n_nodes
        self.n_nodes_pad = ((n_nodes + P - 1) // P) * P
        self.nt = self.n_nodes_pad // P   # node tiles for the table build
        self.split = min(split, self.n_nodes_pad)
        self.elu = elu
        self.wcols = n_h + 2 * heads
        assert self.elem % 64 == 0 and self.elem >= self.n_h + self.heads
        assert self.split % P == 0


class EdgePlan:
    """Host-side edge structures, uniform across cores (SPMD)."""

    def __init__(self, src, dst, cfg: LayerCfg):
        nc_, npc, blocks, split = cfg.n_cores, cfg.nodes_per_core, cfg.blocks, cfg.split
        src = np.asarray(src, dtype=np.int64)
        dst = np.asarray(dst, dtype=np.int64)
        core = dst // npc
        dstl = dst - core * npc
        blk = dstl // P
        dstb = (dstl - blk * P).astype(np.float32)
        grp = (src >= split).astype(np.int64)
        key = (core * blocks + blk) * 2 + grp
        order = np.argsort(key, kind="stable")
        cnt = np.bincount(key, minlength=nc_ * blocks * 2).reshape(nc_, blocks, 2)
        T = -(-cnt.max(axis=0) // P)          # [blocks, 2] tiles per block/group
        T[:, 0] = np.maximum(T[:, 0], 1)      # guarantee psum init per block
        self.T = T
        self.TL = int(T[:, 0].sum())
        self.TH = int(T[:, 1].sum())
        self.T_ALL = self.TL + self.TH
        self.lo_start = np.concatenate([[0], np.cumsum(T[:, 0])])[:-1]
        self.hi_start = np.concatenate([[0], np.cumsum(T[:, 1])])[:-1]
        ptr = np.concatenate([[0], np.cumsum(cnt.reshape(-1))])

        idx_lo = np.zeros((nc_, self.TL * P), np.int16)
        idx_hi = np.zeros((nc_, max(self.TH, 1) * P), np.int16)
        dstc = np.full((nc_, self.T_ALL * P), -1.0, np.float32)
        for c in range(nc_):
            for b in range(blocks):
                for g in range(2):
                    k = (c * blocks + b) * 2 + g
                    e0, e1 = int(ptr[k]), int(ptr[k + 1])
                    if e1 == e0:
                        continue
                    eidx = order[e0:e1]
                    n = e1 - e0
                    if g == 0:
                        base = int(self.lo_start[b]) * P
                        idx_lo[c, base:base + n] = (src[eidx]).astype(np.int16)
                        sbase = base
                    else:
                        base = int(self.hi_start[b]) * P
                        idx_hi[c, base:base + n] = (src[eidx] - split).astype(np.int16)
                        sbase = self.TL * P + base
                    dstc[c, sbase:sbase + n] = dstb[eidx]

        # gather instruction split (1024 idx each, tail partial)
        def gsizes(n_tiles):
            out = []
            rem = n_tiles
            while rem > 0:
                t = min(8, rem)
                out.append(t * P)
                rem -= t
            return out

        self.glo_sizes = gsizes(self.TL)
        self.ghi_sizes = gsizes(self.TH)

        def wrap_idx(flat_all, sizes):
            ng = max(len(sizes), 1)
            out = np.zeros((nc_, P, ng * 64), np.int16)
            for c in range(nc_):
                pos = 0
                for gi, ni in enumerate(sizes):
                    blk16 = flat_all[c, pos:pos + ni].reshape(ni // 16, 16).T
                    out[c, :, gi * 64: gi * 64 + ni // 16] = np.tile(blk16, (8, 1))
                    pos += ni
            return out

        self.idx_lo_w = wrap_idx(idx_lo, self.glo_sizes)
        self.idx_hi_w = wrap_idx(idx_hi, self.ghi_sizes)

        # dstc columns [128, T_ALL]: column t = dst-in-block of the tile's edges
        self.dstc_cols = dstc.reshape(nc_, self.T_ALL, P).transpose(0, 2, 1).copy()

        # dstT2: one row per gather (lo gathers then hi gathers): the gather's
        # edge destinations laid out along the free dim, for the K=1 PE
        # broadcast matmul.
        self.n_gathers = len(self.glo_sizes) + len(self.ghi_sizes)
        self.dstT2 = np.full((nc_, max(self.n_gathers, 1), 1024), -1.0, np.float32)
        gq = 0
        pos = 0
        for ni in self.glo_sizes + self.ghi_sizes:
            self.dstT2[:, gq, 0:ni] = dstc[:, pos:pos + ni]
            pos += ni
            gq += 1

        # per-block tile lists: (stream, stream_tile_idx, abs_t)
        self.block_tiles = []
        for b in range(blocks):
            tl = [("lo", int(self.lo_start[b]) + i, int(self.lo_start[b]) + i)
                  for i in range(int(T[b, 0]))]
            th = [("hi", int(self.hi_start[b]) + i, self.TL + int(self.hi_start[b]) + i)
                  for i in range(int(T[b, 1]))]
            self.block_tiles.append(tl + th)
        # map stream tile -> block
        self.tile_block = {}
        for b, tl in enumerate(self.block_tiles):
            for (s, t, a) in tl:
                self.tile_block[(s, t)] = b


def build_layer(cfg: LayerCfg, plan: EdgePlan):
    nc = bacc.Bacc("TRN2", target_bir_lowering=False, debug=False,
                   num_devices=cfg.n_cores, dynamic_dma_scratch_size=65536,
                   num_swdge_queues=2)
    n_in, n_h, heads, hid = cfg.n_in, cfg.n_h, cfg.heads, cfg.hid
    elem, wcols = cfg.elem, cfg.wcols
    kchunks = n_in // P
    NL = cfg.split
    NH = cfg.n_nodes_pad - cfg.split

    xT = nc.dram_tensor("xT", [n_in, cfg.n_nodes_pad], BF16, kind="ExternalInput").ap()
    xT_own = nc.dram_tensor("xT_own", [n_in, cfg.npad], BF16, kind="ExternalInput").ap()
    Wcat = nc.dram_tensor("Wcat", [P, kchunks * wcols], BF16, kind="ExternalInput").ap()
    bias_rep = nc.dram_tensor("bias_rep", [P, n_h], F32, kind="ExternalInput").ap()
    iota_row = nc.dram_tensor("iota_row", [P, P], BF16, kind="ExternalInput").ap()
    iota_col = nc.dram_tensor("iota_col", [P, 1], F32, kind="ExternalInput").ap()
    ones_row = nc.dram_tensor("ones_row", [1, P], BF16, kind="ExternalInput").ap()
    nglo = max(len(plan.glo_sizes), 1)
    nghi = max(len(plan.ghi_sizes), 1)
    idx_lo = nc.dram_tensor("idx_lo", [P, nglo * 64], I16, kind="ExternalInput").ap()
    idx_hi = nc.dram_tensor("idx_hi", [P, nghi * 64], I16, kind="ExternalInput").ap()
    dstc = nc.dram_tensor("dstc", [P, plan.T_ALL], BF16, kind="ExternalInput").ap()
    dstT2 = nc.dram_tensor("dstT2", [max(plan.n_gathers, 1), 1024], BF16, kind="ExternalInput").ap()
    out_x = nc.dram_tensor("out_x", [cfg.npad, n_h], F32, kind="ExternalOutput").ap()
    dbg = getattr(cfg, "debug", False)
    if dbg:
        dbg_pout = nc.dram_tensor("dbg_pout", [cfg.npad, n_h + heads], F32,
                                  kind="ExternalOutput").ap()
        dbg_gbuf = nc.dram_tensor("dbg_gbuf", [8, P, 8 * elem], F32,
                                  kind="ExternalOutput").ap()
        dbg_wg = nc.dram_tensor("dbg_wg", [8, P, 8 * (n_h + heads)], F32,
                                kind="ExternalOutput").ap()
        dbg_w8 = nc.dram_tensor("dbg_w8", [8, P, 8 * heads], F32,
                                kind="ExternalOutput").ap()
    tab_lo = nc.dram_tensor("tab_lo", [NL, elem], F32).ap()
    tab_hi = nc.dram_tensor("tab_hi", [max(NH, P), elem], F32).ap()

    with tile.TileContext(nc) as tc:
        with tc.tile_pool(name="const", bufs=1) as cpool:
            Wcat_sb = cpool.tile([P, kchunks * wcols], BF16)
            nc.sync.dma_start(out=Wcat_sb[:], in_=Wcat[:])
            bias_sb = cpool.tile([P, n_h], F32)
            nc.sync.dma_start(out=bias_sb[:], in_=bias_rep[:])
            ir_sb = cpool.tile([P, P], BF16)
            nc.sync.dma_start(out=ir_sb[:], in_=iota_row[:])
            ic_sb = cpool.tile([P, 1], F32)
            nc.sync.dma_start(out=ic_sb[:], in_=iota_col[:])
            ones_sb = cpool.tile([1, P], BF16)
            nc.sync.dma_start(out=ones_sb[:], in_=ones_row[:])
            ixlo_sb = cpool.tile([P, nglo * 64], I16)
            nc.sync.dma_start(out=ixlo_sb[:], in_=idx_lo[:])
            ixhi_sb = cpool.tile([P, nghi * 64], I16)
            nc.sync.dma_start(out=ixhi_sb[:], in_=idx_hi[:])
            dstc_sb = cpool.tile([P, plan.T_ALL], BF16)
            nc.sync.dma_start(out=dstc_sb[:], in_=dstc[:])
            er_all = cpool.tile([P, cfg.blocks * heads], BF16)

            # ---- Phase A: projection tables ----
            tab_writes = []
            with tc.tile_pool(name="pa_sb", bufs=3) as apool, \
                 tc.tile_pool(name="pa_ps", bufs=2, space="PSUM") as appool:
                for j in range(cfg.nt):
                    xa = []
                    for k in range(kchunks):
                        t = apool.tile([P, P], BF16, tag=f"x{k}")
                        nc.sync.dma_start(
                            out=t[:], in_=xT[k * P:(k + 1) * P, j * P:(j + 1) * P])
                        xa.append(t)
                    ps = appool.tile([P, wcols], F32, tag="ps")
                    for k in range(kchunks):
                        nc.tensor.matmul(
                            out=ps[:], lhsT=xa[k][:],
                            rhs=Wcat_sb[:, k * wcols:k * wcols + wcols],
                            start=(k == 0), stop=(k == kchunks - 1))
                    row = apool.tile([P, n_h + heads], F32, tag="row")
                    nc.vector.tensor_copy(row[:], ps[:, 0:n_h + heads])
                    if j * P < NL:
                        dst_ap = tab_lo[j * P:(j + 1) * P, 0:n_h + heads]
                    else:
                        jj = j - NL // P
                        dst_ap = tab_hi[jj * P:(jj + 1) * P, 0:n_h + heads]
                    tab_writes.append(nc.sync.dma_start(out=dst_ap, in_=row[:]))
                # er for own nodes -> SBUF resident
                for b in range(cfg.blocks):
                    xa = []
                    for k in range(kchunks):
                        t = apool.tile([P, P], BF16, tag=f"xo{k}")
                        nc.sync.dma_start(
                            out=t[:], in_=xT_own[k * P:(k + 1) * P, b * P:(b + 1) * P])
                        xa.append(t)
                    ps = appool.tile([P, heads], F32, tag="pser")
                    for k in range(kchunks):
                        nc.tensor.matmul(
                            out=ps[:], lhsT=xa[k][:],
                            rhs=Wcat_sb[:, k * wcols + n_h + heads:k * wcols + n_h + 2 * heads],
                            start=(k == 0), stop=(k == kchunks - 1))
                    nc.vector.tensor_copy(er_all[:, b * heads:(b + 1) * heads], ps[:])

            # ---- Phase B: edge processing ----
            # fence: every gather must run after all phase-A table writes
            fence_tile = cpool.tile([1, 1], F32)
            fence = nc.vector.memset(fence_tile[:], 0.0)
            for wi in tab_writes:
                add_dep_helper(fence.ins, wi.ins, True, "gather tables written")
            with tc.tile_pool(name="glo", bufs=3) as glo_pool, \
                 tc.tile_pool(name="ghi", bufs=3) as ghi_pool, \
                 tc.tile_pool(name="sel", bufs=4) as sel_pool, \
                 tc.tile_pool(name="wg", bufs=4) as wg_pool, \
                 tc.tile_pool(name="sw", bufs=4) as sw_pool, \
                 tc.tile_pool(name="ep", bufs=2) as ep_pool, \
                 tc.tile_pool(name="ps_bc", bufs=2, space="PSUM") as bc_pool, \
                 tc.tile_pool(name="ps_er", bufs=2, space="PSUM") as er_pool, \
                 tc.tile_pool(name="ps_out", bufs=2, space="PSUM") as out_pool:

                group_data = {}

                def ensure_gather(strm, gi):
                    if (strm, gi) in group_data:
                        return group_data[(strm, gi)]
                    if strm == "lo":
                        ni = plan.glo_sizes[gi]
                        gq = gi
                        pool_, tab, ixsb, qn = glo_pool, tab_lo, ixlo_sb, 0
                    else:
                        ni = plan.ghi_sizes[gi]
                        gq = len(plan.glo_sizes) + gi
                        pool_, tab, ixsb, qn = ghi_pool, tab_hi, ixhi_sb, 1
                    ngt = ni // P
                    buf = pool_.tile([P, 8, elem], F32, tag="g" + strm)
                    gins = nc.gpsimd.dma_gather(
                        buf[:, 0:ngt, :], tab[:],
                        ixsb[:, gi * 64:gi * 64 + ni // 16],
                        ni, ni, elem, queue_num=qn)
                    add_dep_helper(gins.ins, fence.ins, True, "gather after fence")
                    if dbg and gq < 8:
                        nc.sync.dma_start(out=dbg_gbuf[gq, :, :],
                                          in_=buf[:].rearrange("p a b -> p (a b)"))
                    # broadcast dst rows for this gather: psum_bc[n, e] = dst[e]
                    dr = sw_pool.tile([1, 1024], BF16, tag="dr")
                    nc.sync.dma_start(out=dr[:, 0:ni], in_=dstT2[gq:gq + 1, 0:ni])
                    bc = bc_pool.tile([P, 1024], F32, tag="bc")
                    for h in range(0, ni, 512):
                        w = min(512, ni - h)
                        nc.tensor.matmul(
                            out=bc[:, h:h + w], lhsT=ones_sb[:],
                            rhs=dr[:, h:h + w],
                            start=True, stop=True)
                    selt8 = sel_pool.tile([P, 1024], BF16, tag="selt")
                    nc.vector.tensor_scalar(
                        selt8[:, 0:ni], bc[:, 0:ni], ic_sb[:], None, AluOp.is_equal)
                    # SEL8: one-hot along free (node) axis per tile
                    sel8 = sel_pool.tile([P, 1024], BF16, tag="sel8")
                    t0_abs = plan.abs_base[(strm, gi)]
                    in1 = dstc_sb[:, t0_abs:t0_abs + ngt].unsqueeze(2).to_broadcast([P, ngt, P])
                    nc.vector.tensor_tensor(
                        out=sel8[:, 0:ni].rearrange("p (t n) -> p t n", t=ngt),
                        in0=ir_sb[:].unsqueeze(1).to_broadcast([P, ngt, P]),
                        in1=in1, op=AluOp.is_equal)
                    # er per edge: er8[:, r*heads:...] = selt8_r^T-matmul er_blk
                    er8 = er_pool.tile([P, 8 * heads], F32, tag="er8")
                    for r in range(ngt):
                        bb = plan.tile_block[(strm, gi * 8 + r)]
                        nc.tensor.matmul(
                            out=er8[:, r * heads:(r + 1) * heads],
                            lhsT=selt8[:, r * P:(r + 1) * P],
                            rhs=er_all[:, bb * heads:(bb + 1) * heads],
                            start=True, stop=True)
                    # s = el + er ; w = exp(leaky_relu(s))
                    s8 = sw_pool.tile([P, 8 * heads], F32, tag="s8")
                    nc.vector.tensor_tensor(
                        out=s8[:, 0:ngt * heads].rearrange("p (t h) -> p t h", t=ngt),
                        in0=er8[:, 0:ngt * heads].rearrange("p (t h) -> p t h", t=ngt),
                        in1=buf[:, 0:ngt, n_h:n_h + heads], op=AluOp.add)
                    lr8 = sw_pool.tile([P, 8 * heads], F32, tag="lr8")
                    nc.vector.scalar_tensor_tensor(
                        lr8[:, 0:ngt * heads], s8[:, 0:ngt * heads], NEG_SLOPE,
                        s8[:, 0:ngt * heads], AluOp.mult, AluOp.max)
                    w8 = sw_pool.tile([P, 8 * heads], F32, tag="w8")
                    nc.scalar.activation(w8[:, 0:ngt * heads], lr8[:, 0:ngt * heads],
                                         ActFn.Exp)
                    # WG8 = [h * w | w] per tile (single 264-wide matmul rhs)
                    nf = n_h + heads
                    wg8 = wg_pool.tile([P, 8 * nf], BF16, tag="wg8")
                    wg8v = wg8[:].rearrange("p (t f) -> p t f", f=nf)
                    nc.vector.tensor_tensor(
                        out=wg8v[:, 0:ngt, 0:n_h].rearrange(
                            "p t (h d) -> p t h d", h=heads),
                        in0=buf[:, 0:ngt, 0:n_h].rearrange(
                            "p t (h d) -> p t h d", h=heads),
                        in1=w8[:, 0:ngt * heads].rearrange(
                            "p (t h) -> p t h", t=ngt).unsqueeze(3).to_broadcast(
                            [P, ngt, heads, hid]),
                        op=AluOp.mult)
                    nc.vector.tensor_copy(
                        wg8v[:, 0:ngt, n_h:nf],
                        w8[:, 0:ngt * heads].rearrange("p (t h) -> p t h", t=ngt))
                    if dbg and gq < 8:
                        nc.sync.dma_start(out=dbg_wg[gq, :, :], in_=wg8[:])
                        nc.sync.dma_start(out=dbg_w8[gq, :, :], in_=w8[:])
                    group_data[(strm, gi)] = (sel8, wg8, w8)
                    return group_data[(strm, gi)]

                for b in range(cfg.blocks):
                    tiles = plan.block_tiles[b]
                    pout = out_pool.tile([P, n_h + heads], F32, tag="pout")
                    for i, (strm, t, abs_t) in enumerate(tiles):
                        gi, r = divmod(t, 8)
                        sel8, wg8, w8 = ensure_gather(strm, gi)
                        st = (i == 0)
                        sp = (i == len(tiles) - 1)
                        nf = n_h + heads
                        nc.tensor.matmul(
                            out=pout[:], lhsT=sel8[:, r * P:(r + 1) * P],
                            rhs=wg8[:, r * nf:(r + 1) * nf], start=st, stop=sp)
                    # epilogue
                    if dbg:
                        dtile = ep_pool.tile([P, n_h + heads], F32, tag="dbg")
                        nc.scalar.copy(dtile[:], pout[:])
                        nc.sync.dma_start(out=dbg_pout[b * P:(b + 1) * P, :], in_=dtile[:])
                    dn = ep_pool.tile([P, heads], F32, tag="dn")
                    nc.vector.tensor_scalar_add(dn[:], pout[:, n_h:n_h + heads], 1e-30)
                    rec = ep_pool.tile([P, heads], F32, tag="rec")
                    nc.vector.reciprocal(rec[:], dn[:])
                    ox = ep_pool.tile([P, n_h], F32, tag="ox")
                    nc.vector.tensor_tensor(
                        out=ox[:].rearrange("p (h d) -> p h d", h=heads),
                        in0=pout[:, 0:n_h].rearrange("p (h d) -> p h d", h=heads),
                        in1=rec[:].unsqueeze(2).to_broadcast([P, heads, hid]),
                        op=AluOp.mult)
                    nc.vector.tensor_add(ox[:], ox[:], bias_sb[:])
                    if cfg.elu:
                        mn = ep_pool.tile([P, n_h], F32, tag="mn")
                        nc.vector.tensor_scalar_min(mn[:], ox[:], 0.0)
                        em = ep_pool.tile([P, n_h], F32, tag="em")
                        nc.scalar.activation(em[:], mn[:], ActFn.Exp)
                        mx = ep_pool.tile([P, n_h], F32, tag="mx")
                        nc.vector.tensor_scalar_max(mx[:], ox[:], 0.0)
                        nc.vector.tensor_add(ox[:], mx[:], em[:])
                        nc.vector.tensor_scalar_add(ox[:], ox[:], -1.0)
                    nc.sync.dma_start(out=out_x[b * P:(b + 1) * P, :], in_=ox[:])

    nc.compile()
    return nc


def _prep_layer_inputs(cfg: LayerCfg, plan: EdgePlan, x_full, W, attn_l, attn_r, bias):
    """x_full: [n_nodes, n_in] fp32. Returns list of per-core input dicts."""
    n_in, n_h, heads, hid = cfg.n_in, cfg.n_h, cfg.heads, cfg.hid
    wcols = cfg.wcols
    # fold attention vectors: Wl = W @ blockdiag(attn_l)
    Bl = np.zeros((n_h, heads), np.float32)
    Br = np.zeros((n_h, heads), np.float32)
    for h in range(heads):
        Bl[h * hid:(h + 1) * hid, h] = attn_l[h]
        Br[h * hid:(h + 1) * hid, h] = attn_r[h]
    Wc = np.concatenate([W, W @ Bl, W @ Br], axis=1).astype(np.float32)  # [n_in, wcols]
    kchunks = n_in // P
    Wcat_host = Wc.reshape(kchunks, P, wcols).transpose(1, 0, 2).reshape(P, kchunks * wcols)
    Wcat_host = np.ascontiguousarray(Wcat_host).astype(BF16_NP)

    xT = np.zeros((n_in, cfg.n_nodes_pad), BF16_NP)
    xT[:, 0:cfg.n_nodes] = np.ascontiguousarray(x_full.T)

    bias_r = np.ascontiguousarray(np.tile(bias.reshape(1, n_h), (P, 1)).astype(np.float32))
    iota_row = np.ascontiguousarray(
        np.tile(np.arange(P, dtype=np.float32).reshape(1, P), (P, 1))).astype(BF16_NP)
    iota_col = np.ascontiguousarray(np.arange(P, dtype=np.float32).reshape(P, 1))
    ones_row = np.ones((1, P), BF16_NP)

    ins = []
    npc = cfg.nodes_per_core
    for c in range(cfg.n_cores):
        xo = np.zeros((n_in, cfg.npad), BF16_NP)
        lo = c * npc
        hi = min((c + 1) * npc, cfg.n_nodes)
        xo[:, 0:hi - lo] = x_full[lo:hi].T
        ins.append({
            "xT": xT,
            "xT_own": np.ascontiguousarray(xo),
            "Wcat": Wcat_host,
            "bias_rep": bias_r,
            "iota_row": iota_row,
            "iota_col": iota_col,
            "ones_row": ones_row,
            "idx_lo": np.ascontiguousarray(plan.idx_lo_w[c]),
            "idx_hi": np.ascontiguousarray(plan.idx_hi_w[c]),
            "dstc": np.ascontiguousarray(plan.dstc_cols[c]).astype(BF16_NP),
            "dstT2": np.ascontiguousarray(plan.dstT2[c]).astype(BF16_NP),
        })
    return ins


def run_gat(emb, src, dst, W1, attn_l1, attn_r1, bias1, W2, attn_l2, attn_r2, bias2,
            n_nodes=N_NODES, split=32768, trace=False, tmpdir=None):
    emb = np.asarray(emb, np.float32)
    n_in = emb.shape[1]
    cfg1 = LayerCfg(n_in, HEADS * HID, HEADS, 320, n_nodes, N_CORES, split, elu=True)
    cfg2 = LayerCfg(HEADS * HID, OUT_DIM, 1, 64, n_nodes, N_CORES, split, elu=False)
    plan = EdgePlan(src, dst, cfg1)
    # annotate abs base col per gather group (for SEL8 build)
    plan.abs_base = {}
    pos = 0
    for gi, ni in enumerate(plan.glo_sizes):
        plan.abs_base[("lo", gi)] = pos
        pos += ni // P
    for gi, ni in enumerate(plan.ghi_sizes):
        plan.abs_base[("hi", gi)] = pos
        pos += ni // P

    nc1 = build_layer(cfg1, plan)
    ins1 = _prep_layer_inputs(cfg1, plan, emb, np.asarray(W1, np.float32),
                              np.asarray(attn_l1, np.float32),
                              np.asarray(attn_r1, np.float32),
                              np.asarray(bias1, np.float32))
    res1 = run_bass_kernel_spmd(nc1, ins1, list(range(N_CORES)), trace=trace,
                                tmpdir=None if tmpdir is None else tmpdir + "_l1")
    npc = cfg1.nodes_per_core
    x2 = np.concatenate(
        [res1.results[c]["out_x"][0:min(npc, n_nodes - c * npc)] for c in range(N_CORES)],
        axis=0)  # [n_nodes, 256]

    nc2 = build_layer(cfg2, plan)
    ins2 = _prep_layer_inputs(cfg2, plan, x2, np.asarray(W2, np.float32),
                              np.asarray(attn_l2, np.float32).reshape(1, OUT_DIM),
                              np.asarray(attn_r2, np.float32).reshape(1, OUT_DIM),
                              np.asarray(bias2, np.float32))
    res2 = run_bass_kernel_spmd(nc2, ins2, list(range(N_CORES)), trace=trace,
                                tmpdir=None if tmpdir is None else tmpdir + "_l2")
    out = np.concatenate(
        [res2.results[c]["out_x"][0:min(npc, n_nodes - c * npc)] for c in range(N_CORES)],
        axis=0)
    exec_ns = [res1.exec_time_ns, res2.exec_time_ns]
    return out.astype(np.float32), exec_ns


def kernel(emb, src, dst, W1, attn_l1, attn_r1, bias1, W2, attn_l2, attn_r2, bias2):
    out, _ = run_gat(emb, src, dst, W1, attn_l1, attn_r1, bias1,
                     W2, attn_l2, attn_r2, bias2)
    return out


# revision 17
# speedup vs baseline: 1.5133x; 1.5133x over previous
"""Trainium2 Bass kernel for a 2-layer GAT node classifier (SPMD over 8 NeuronCores).

Strategy (per layer):
  - Replicated dense phase: every core computes the full projection table
    H'[n] = [x @ W | x @ (W B_l)] (h plus the per-head left-attention dot),
    written to per-core HBM gather tables. The right-attention dot er is kept
    only for the core's own destination-node range, resident in SBUF.
  - Edge phase: destination nodes are sharded contiguously across cores
    (6250 per core). Per core, edges sorted by destination, grouped into
    128-node destination blocks, padded to 128-edge tiles, and split into
    "lo"/"hi" source groups so the int16 gather indices can address the
    whole 50048-row table via two base tables.
  - Per 1024-edge gather group (one SWDGE dma_gather instruction): build
    selection matrices SEL (edges x nodes one-hot by destination) and its
    transpose via iota-compare + a K=1 PE broadcast matmul, compute
    unnormalized attention weights w = exp(leaky_relu(el[src] + er[dst])),
    and accumulate [sum w*h[src] | sum w] per destination block with PE
    matmuls (exact segment-sum via one-hot matmul). The edge softmax
    denominator is applied per node at the end (alpha never materialized;
    segment-max is skipped — scores are O(1) so exp is safe).
  - Epilogue per block: divide by denominator, add bias, (layer 1: ELU),
    write the core's output slice.

The host only does index preprocessing (graph partition / sort / padding),
weight repacking (folding attention vectors into the weight matrix:
W @ blockdiag(attn)), transposes of inputs, and concatenation of outputs.
All floating-point compute on the 800k edges / 50k nodes runs on device.
"""

import math
import numpy as np
import ml_dtypes

BF16_NP = ml_dtypes.bfloat16

import concourse.bacc as bacc
import concourse.tile as tile
from concourse.tile_rust import add_dep_helper
from concourse import mybir
from concourse.bass_utils import run_bass_kernel_spmd

P = 128
N_CORES = 8
AluOp = mybir.AluOpType
ActFn = mybir.ActivationFunctionType
F32 = mybir.dt.float32
BF16 = mybir.dt.bfloat16
I16 = mybir.dt.int16

# Problem constants (nn_GAT_Node_Classifier)
N_NODES = 50000
N_EDGES = 800000
IN_DIM = 256
HID = 32
HEADS = 8
OUT_DIM = 16
NEG_SLOPE = 0.2


class LayerCfg:
    def __init__(self, n_in, n_h, heads, elem, n_nodes, n_cores, split, elu):
        self.n_in = n_in              # input feature dim (must be mult of 128)
        self.n_h = n_h                # heads * hid
        self.heads = heads
        self.hid = n_h // heads
        self.elem = elem              # gather row floats (>= n_h + heads, 64B mult)
        self.n_cores = n_cores
        self.nodes_per_core = n_nodes // n_cores
        self.blocks = (self.nodes_per_core + P - 1) // P
        self.npad = self.blocks * P
        self.n_nodes = n_nodes
        self.n_nodes_pad = ((n_nodes + P - 1) // P) * P
        self.nt = self.n_nodes_pad // P   # node tiles for the table build
        self.split = min(split, self.n_nodes_pad)
        self.elu = elu
        self.wcols = n_h + 2 * heads
        assert self.elem % 64 == 0 and self.elem >= self.n_h + self.heads
        assert self.split % P == 0


class EdgePlan:
    """Host-side edge structures, uniform across cores (SPMD)."""

    def __init__(self, src, dst, cfg: LayerCfg):
        nc_, npc, blocks, split = cfg.n_cores, cfg.nodes_per_core, cfg.blocks, cfg.split
        src = np.asarray(src, dtype=np.int64)
        dst = np.asarray(dst, dtype=np.int64)
        core = dst // npc
        dstl = dst - core * npc
        blk = dstl // P
        dstb = (dstl - blk * P).astype(np.float32)
        grp = (src >= split).astype(np.int64)
        key = (core * blocks + blk) * 2 + grp
        order = np.argsort(key, kind="stable")
        cnt = np.bincount(key, minlength=nc_ * blocks * 2).reshape(nc_, blocks, 2)
        T = -(-cnt.max(axis=0) // P)          # [blocks, 2] tiles per block/group
        T[:, 0] = np.maximum(T[:, 0], 1)      # guarantee psum init per block
        self.T = T
        self.TL = int(T[:, 0].sum())
        self.TH = int(T[:, 1].sum())
        self.T_ALL = self.TL + self.TH
        self.lo_start = np.concatenate([[0], np.cumsum(T[:, 0])])[:-1]
        self.hi_start = np.concatenate([[0], np.cumsum(T[:, 1])])[:-1]
        ptr = np.concatenate([[0], np.cumsum(cnt.reshape(-1))])

        idx_lo = np.zeros((nc_, self.TL * P), np.int16)
        idx_hi = np.zeros((nc_, max(self.TH, 1) * P), np.int16)
        dstc = np.full((nc_, self.T_ALL * P), -1.0, np.float32)
        for c in range(nc_):
            for b in range(blocks):
                for g in range(2):
                    k = (c * blocks + b) * 2 + g
                    e0, e1 = int(ptr[k]), int(ptr[k + 1])
                    if e1 == e0:
                        continue
                    eidx = order[e0:e1]
                    n = e1 - e0
                    if g == 0:
                        base = int(self.lo_start[b]) * P
                        idx_lo[c, base:base + n] = (src[eidx]).astype(np.int16)
                        sbase = base
                    else:
                        base = int(self.hi_start[b]) * P
                        idx_hi[c, base:base + n] = (src[eidx] - split).astype(np.int16)
                        sbase = self.TL * P + base
                    dstc[c, sbase:sbase + n] = dstb[eidx]

        # gather instruction split (1024 idx each, tail partial)
        def gsizes(n_tiles):
            out = []
            rem = n_tiles
            while rem > 0:
                t = min(8, rem)
                out.append(t * P)
                rem -= t
            return out

        self.glo_sizes = gsizes(self.TL)
        self.ghi_sizes = gsizes(self.TH)

        def wrap_idx(flat_all, sizes):
            ng = max(len(sizes), 1)
            out = np.zeros((nc_, P, ng * 64), np.int16)
            for c in range(nc_):
                pos = 0
                for gi, ni in enumerate(sizes):
                    blk16 = flat_all[c, pos:pos + ni].reshape(ni // 16, 16).T
                    out[c, :, gi * 64: gi * 64 + ni // 16] = np.tile(blk16, (8, 1))
                    pos += ni
            return out

        self.idx_lo_w = wrap_idx(idx_lo, self.glo_sizes)
        self.idx_hi_w = wrap_idx(idx_hi, self.ghi_sizes)

        # dstc columns [128, T_ALL]: column t = dst-in-block of the tile's edges
        self.dstc_cols = dstc.reshape(nc_, self.T_ALL, P).transpose(0, 2, 1).copy()

        # dstT2: one row per gather (lo gathers then hi gathers): the gather's
        # edge destinations laid out along the free dim, for the K=1 PE
        # broadcast matmul.
        self.n_gathers = len(self.glo_sizes) + len(self.ghi_sizes)
        self.dstT2 = np.full((nc_, max(self.n_gathers, 1), 1024), -1.0, np.float32)
        gq = 0
        pos = 0
        for ni in self.glo_sizes + self.ghi_sizes:
            self.dstT2[:, gq, 0:ni] = dstc[:, pos:pos + ni]
            pos += ni
            gq += 1

        # per-block tile lists: (stream, stream_tile_idx, abs_t)
        self.block_tiles = []
        for b in range(blocks):
            tl = [("lo", int(self.lo_start[b]) + i, int(self.lo_start[b]) + i)
                  for i in range(int(T[b, 0]))]
            th = [("hi", int(self.hi_start[b]) + i, self.TL + int(self.hi_start[b]) + i)
                  for i in range(int(T[b, 1]))]
            self.block_tiles.append(tl + th)
        # map stream tile -> block
        self.tile_block = {}
        for b, tl in enumerate(self.block_tiles):
            for (s, t, a) in tl:
                self.tile_block[(s, t)] = b


def build_layer(cfg: LayerCfg, plan: EdgePlan):
    nc = bacc.Bacc("TRN2", target_bir_lowering=False, debug=False,
                   num_devices=cfg.n_cores, dynamic_dma_scratch_size=65536,
                   num_swdge_queues=4)
    n_in, n_h, heads, hid = cfg.n_in, cfg.n_h, cfg.heads, cfg.hid
    elem, wcols = cfg.elem, cfg.wcols
    kchunks = n_in // P
    NL = cfg.split
    NH = cfg.n_nodes_pad - cfg.split

    xT = nc.dram_tensor("xT", [n_in, cfg.n_nodes_pad], BF16, kind="ExternalInput").ap()
    xT_own = nc.dram_tensor("xT_own", [n_in, cfg.npad], BF16, kind="ExternalInput").ap()
    Wcat = nc.dram_tensor("Wcat", [P, kchunks * wcols], BF16, kind="ExternalInput").ap()
    bias_rep = nc.dram_tensor("bias_rep", [P, n_h], F32, kind="ExternalInput").ap()
    iota_row = nc.dram_tensor("iota_row", [P, P], BF16, kind="ExternalInput").ap()
    iota_col = nc.dram_tensor("iota_col", [P, 1], F32, kind="ExternalInput").ap()
    ones_row = nc.dram_tensor("ones_row", [1, P], BF16, kind="ExternalInput").ap()
    nglo = max(len(plan.glo_sizes), 1)
    nghi = max(len(plan.ghi_sizes), 1)
    idx_lo = nc.dram_tensor("idx_lo", [P, nglo * 64], I16, kind="ExternalInput").ap()
    idx_hi = nc.dram_tensor("idx_hi", [P, nghi * 64], I16, kind="ExternalInput").ap()
    dstc = nc.dram_tensor("dstc", [P, plan.T_ALL], BF16, kind="ExternalInput").ap()
    dstT2 = nc.dram_tensor("dstT2", [max(plan.n_gathers, 1), 1024], BF16, kind="ExternalInput").ap()
    out_x = nc.dram_tensor("out_x", [cfg.npad, n_h], F32, kind="ExternalOutput").ap()
    dbg = getattr(cfg, "debug", False)
    if dbg:
        dbg_pout = nc.dram_tensor("dbg_pout", [cfg.npad, n_h + heads], F32,
                                  kind="ExternalOutput").ap()
        dbg_gbuf = nc.dram_tensor("dbg_gbuf", [8, P, 8 * elem], F32,
                                  kind="ExternalOutput").ap()
        dbg_wg = nc.dram_tensor("dbg_wg", [8, P, 8 * (n_h + heads)], F32,
                                kind="ExternalOutput").ap()
        dbg_w8 = nc.dram_tensor("dbg_w8", [8, P, 8 * heads], F32,
                                kind="ExternalOutput").ap()
    tab_lo = nc.dram_tensor("tab_lo", [NL, elem], F32).ap()
    tab_hi = nc.dram_tensor("tab_hi", [max(NH, P), elem], F32).ap()

    with tile.TileContext(nc) as tc:
        with tc.tile_pool(name="const", bufs=1) as cpool:
            Wcat_sb = cpool.tile([P, kchunks * wcols], BF16)
            nc.sync.dma_start(out=Wcat_sb[:], in_=Wcat[:])
            bias_sb = cpool.tile([P, n_h], F32)
            nc.sync.dma_start(out=bias_sb[:], in_=bias_rep[:])
            ir_sb = cpool.tile([P, P], BF16)
            nc.sync.dma_start(out=ir_sb[:], in_=iota_row[:])
            ic_sb = cpool.tile([P, 1], F32)
            nc.sync.dma_start(out=ic_sb[:], in_=iota_col[:])
            ones_sb = cpool.tile([1, P], BF16)
            nc.sync.dma_start(out=ones_sb[:], in_=ones_row[:])
            ixlo_sb = cpool.tile([P, nglo * 64], I16)
            nc.sync.dma_start(out=ixlo_sb[:], in_=idx_lo[:])
            ixhi_sb = cpool.tile([P, nghi * 64], I16)
            nc.sync.dma_start(out=ixhi_sb[:], in_=idx_hi[:])
            dstc_sb = cpool.tile([P, plan.T_ALL], BF16)
            nc.sync.dma_start(out=dstc_sb[:], in_=dstc[:])
            er_all = cpool.tile([P, cfg.blocks * heads], BF16)

            # ---- Phase A: projection tables ----
            # batched: GT node tiles per DMA group to amortize DMA issue cost
            GT = 4
            tab_writes = []
            with tc.tile_pool(name="pa_sb", bufs=3) as apool, \
                 tc.tile_pool(name="pa_ps", bufs=4, space="PSUM") as appool:
                assert NL % (GT * P) == 0
                for j0 in range(0, cfg.nt, GT):
                    gsz = min(GT, cfg.nt - j0)
                    xa = []
                    for k in range(kchunks):
                        t = apool.tile([P, GT * P], BF16, tag=f"x{k}")
                        nc.sync.dma_start(
                            out=t[:, 0:gsz * P],
                            in_=xT[k * P:(k + 1) * P, j0 * P:(j0 + gsz) * P])
                        xa.append(t)
                    stage = apool.tile([P, GT, n_h + heads], F32, tag="stage")
                    for jj in range(gsz):
                        ps = appool.tile([P, wcols], F32, tag="ps")
                        for k in range(kchunks):
                            nc.tensor.matmul(
                                out=ps[:], lhsT=xa[k][:, jj * P:(jj + 1) * P],
                                rhs=Wcat_sb[:, k * wcols:k * wcols + wcols],
                                start=(k == 0), stop=(k == kchunks - 1))
                        nc.vector.tensor_copy(stage[:, jj, :], ps[:, 0:n_h + heads])
                    if j0 * P < NL:
                        dst_ap = tab_lo[j0 * P:(j0 + gsz) * P, 0:n_h + heads]
                    else:
                        r0 = j0 * P - NL
                        dst_ap = tab_hi[r0:r0 + gsz * P, 0:n_h + heads]
                    dst_ap = dst_ap.rearrange("(g p) c -> p g c", p=P)
                    tab_writes.append(
                        nc.sync.dma_start(out=dst_ap, in_=stage[:, 0:gsz, :]))
                # er for own nodes -> SBUF resident
                for b0 in range(0, cfg.blocks, GT):
                    gsz = min(GT, cfg.blocks - b0)
                    xa = []
                    for k in range(kchunks):
                        t = apool.tile([P, GT * P], BF16, tag=f"xo{k}")
                        nc.sync.dma_start(
                            out=t[:, 0:gsz * P],
                            in_=xT_own[k * P:(k + 1) * P, b0 * P:(b0 + gsz) * P])
                        xa.append(t)
                    for jj in range(gsz):
                        b = b0 + jj
                        ps = appool.tile([P, heads], F32, tag="pser")
                        for k in range(kchunks):
                            nc.tensor.matmul(
                                out=ps[:], lhsT=xa[k][:, jj * P:(jj + 1) * P],
                                rhs=Wcat_sb[:, k * wcols + n_h + heads:k * wcols + n_h + 2 * heads],
                                start=(k == 0), stop=(k == kchunks - 1))
                        nc.vector.tensor_copy(er_all[:, b * heads:(b + 1) * heads], ps[:])

            # ---- Phase B: edge processing ----
            # fence: every gather must run after all phase-A table writes
            fence_tile = cpool.tile([1, 1], F32)
            fence = nc.vector.memset(fence_tile[:], 0.0)
            for wi in tab_writes:
                add_dep_helper(fence.ins, wi.ins, True, "gather tables written")
            with tc.tile_pool(name="glo", bufs=3) as glo_pool, \
                 tc.tile_pool(name="ghi", bufs=3) as ghi_pool, \
                 tc.tile_pool(name="sel", bufs=4) as sel_pool, \
                 tc.tile_pool(name="wg", bufs=4) as wg_pool, \
                 tc.tile_pool(name="sw", bufs=4) as sw_pool, \
                 tc.tile_pool(name="ep", bufs=2) as ep_pool, \
                 tc.tile_pool(name="ps_bc", bufs=2, space="PSUM") as bc_pool, \
                 tc.tile_pool(name="ps_er", bufs=2, space="PSUM") as er_pool, \
                 tc.tile_pool(name="ps_out", bufs=2, space="PSUM") as out_pool:

                group_data = {}

                def ensure_gather(strm, gi):
                    if (strm, gi) in group_data:
                        return group_data[(strm, gi)]
                    if strm == "lo":
                        ni = plan.glo_sizes[gi]
                        gq = gi
                        pool_, tab, ixsb = glo_pool, tab_lo, ixlo_sb
                    else:
                        ni = plan.ghi_sizes[gi]
                        gq = len(plan.glo_sizes) + gi
                        pool_, tab, ixsb = ghi_pool, tab_hi, ixhi_sb
                    ngt = ni // P
                    buf = pool_.tile([P, 8, elem], F32, tag="g" + strm)
                    gins = nc.gpsimd.dma_gather(
                        buf[:, 0:ngt, :], tab[:],
                        ixsb[:, gi * 64:gi * 64 + ni // 16],
                        ni, ni, elem, queue_num=gq % 4)
                    add_dep_helper(gins.ins, fence.ins, True, "gather after fence")
                    if dbg and gq < 8:
                        nc.sync.dma_start(out=dbg_gbuf[gq, :, :],
                                          in_=buf[:].rearrange("p a b -> p (a b)"))
                    # broadcast dst rows for this gather: psum_bc[n, e] = dst[e]
                    dr = sw_pool.tile([1, 1024], BF16, tag="dr")
                    nc.sync.dma_start(out=dr[:, 0:ni], in_=dstT2[gq:gq + 1, 0:ni])
                    bc = bc_pool.tile([P, 1024], F32, tag="bc")
                    for h in range(0, ni, 512):
                        w = min(512, ni - h)
                        nc.tensor.matmul(
                            out=bc[:, h:h + w], lhsT=ones_sb[:],
                            rhs=dr[:, h:h + w],
                            start=True, stop=True)
                    selt8 = sel_pool.tile([P, 1024], BF16, tag="selt")
                    nc.vector.tensor_scalar(
                        selt8[:, 0:ni], bc[:, 0:ni], ic_sb[:], None, AluOp.is_equal)
                    # SEL8: one-hot along free (node) axis per tile
                    sel8 = sel_pool.tile([P, 1024], BF16, tag="sel8")
                    t0_abs = plan.abs_base[(strm, gi)]
                    in1 = dstc_sb[:, t0_abs:t0_abs + ngt].unsqueeze(2).to_broadcast([P, ngt, P])
                    nc.vector.tensor_tensor(
                        out=sel8[:, 0:ni].rearrange("p (t n) -> p t n", t=ngt),
                        in0=ir_sb[:].unsqueeze(1).to_broadcast([P, ngt, P]),
                        in1=in1, op=AluOp.is_equal)
                    # er per edge: er8[:, r*heads:...] = selt8_r^T-matmul er_blk
                    er8 = er_pool.tile([P, 8 * heads], F32, tag="er8")
                    for r in range(ngt):
                        bb = plan.tile_block[(strm, gi * 8 + r)]
                        nc.tensor.matmul(
                            out=er8[:, r * heads:(r + 1) * heads],
                            lhsT=selt8[:, r * P:(r + 1) * P],
                            rhs=er_all[:, bb * heads:(bb + 1) * heads],
                            start=True, stop=True)
                    # s = el + er ; w = exp(leaky_relu(s))
                    s8 = sw_pool.tile([P, 8 * heads], F32, tag="s8")
                    nc.vector.tensor_tensor(
                        out=s8[:, 0:ngt * heads].rearrange("p (t h) -> p t h", t=ngt),
                        in0=er8[:, 0:ngt * heads].rearrange("p (t h) -> p t h", t=ngt),
                        in1=buf[:, 0:ngt, n_h:n_h + heads], op=AluOp.add)
                    lr8 = sw_pool.tile([P, 8 * heads], F32, tag="lr8")
                    nc.vector.scalar_tensor_tensor(
                        lr8[:, 0:ngt * heads], s8[:, 0:ngt * heads], NEG_SLOPE,
                        s8[:, 0:ngt * heads], AluOp.mult, AluOp.max)
                    w8 = sw_pool.tile([P, 8 * heads], F32, tag="w8")
                    nc.scalar.activation(w8[:, 0:ngt * heads], lr8[:, 0:ngt * heads],
                                         ActFn.Exp)
                    # WG8 = [h * w | w] per tile (single 264-wide matmul rhs)
                    nf = n_h + heads
                    wg8 = wg_pool.tile([P, 8 * nf], BF16, tag="wg8")
                    wg8v = wg8[:].rearrange("p (t f) -> p t f", f=nf)
                    nc.vector.tensor_tensor(
                        out=wg8v[:, 0:ngt, 0:n_h].rearrange(
                            "p t (h d) -> p t h d", h=heads),
                        in0=buf[:, 0:ngt, 0:n_h].rearrange(
                            "p t (h d) -> p t h d", h=heads),
                        in1=w8[:, 0:ngt * heads].rearrange(
                            "p (t h) -> p t h", t=ngt).unsqueeze(3).to_broadcast(
                            [P, ngt, heads, hid]),
                        op=AluOp.mult)
                    nc.vector.tensor_copy(
                        wg8v[:, 0:ngt, n_h:nf],
                        w8[:, 0:ngt * heads].rearrange("p (t h) -> p t h", t=ngt))
                    if dbg and gq < 8:
                        nc.sync.dma_start(out=dbg_wg[gq, :, :], in_=wg8[:])
                        nc.sync.dma_start(out=dbg_w8[gq, :, :], in_=w8[:])
                    group_data[(strm, gi)] = (sel8, wg8, w8)
                    return group_data[(strm, gi)]

                for b in range(cfg.blocks):
                    tiles = plan.block_tiles[b]
                    pout = out_pool.tile([P, n_h + heads], F32, tag="pout")
                    for i, (strm, t, abs_t) in enumerate(tiles):
                        gi, r = divmod(t, 8)
                        sel8, wg8, w8 = ensure_gather(strm, gi)
                        st = (i == 0)
                        sp = (i == len(tiles) - 1)
                        nf = n_h + heads
                        nc.tensor.matmul(
                            out=pout[:], lhsT=sel8[:, r * P:(r + 1) * P],
                            rhs=wg8[:, r * nf:(r + 1) * nf], start=st, stop=sp)
                    # epilogue
                    if dbg:
                        dtile = ep_pool.tile([P, n_h + heads], F32, tag="dbg")
                        nc.scalar.copy(dtile[:], pout[:])
                        nc.sync.dma_start(out=dbg_pout[b * P:(b + 1) * P, :], in_=dtile[:])
                    dn = ep_pool.tile([P, heads], F32, tag="dn")
                    nc.vector.tensor_scalar_add(dn[:], pout[:, n_h:n_h + heads], 1e-30)
                    rec = ep_pool.tile([P, heads], F32, tag="rec")
                    nc.vector.reciprocal(rec[:], dn[:])
                    ox = ep_pool.tile([P, n_h], F32, tag="ox")
                    nc.vector.tensor_tensor(
                        out=ox[:].rearrange("p (h d) -> p h d", h=heads),
                        in0=pout[:, 0:n_h].rearrange("p (h d) -> p h d", h=heads),
                        in1=rec[:].unsqueeze(2).to_broadcast([P, heads, hid]),
                        op=AluOp.mult)
                    nc.vector.tensor_add(ox[:], ox[:], bias_sb[:])
                    if cfg.elu:
                        mn = ep_pool.tile([P, n_h], F32, tag="mn")
                        nc.vector.tensor_scalar_min(mn[:], ox[:], 0.0)
                        em = ep_pool.tile([P, n_h], F32, tag="em")
                        nc.scalar.activation(em[:], mn[:], ActFn.Exp)
                        mx = ep_pool.tile([P, n_h], F32, tag="mx")
                        nc.vector.tensor_scalar_max(mx[:], ox[:], 0.0)
                        nc.vector.tensor_add(ox[:], mx[:], em[:])
                        nc.vector.tensor_scalar_add(ox[:], ox[:], -1.0)
                    nc.sync.dma_start(out=out_x[b * P:(b + 1) * P, :], in_=ox[:])

    nc.compile()
    return nc


def _prep_layer_inputs(cfg: LayerCfg, plan: EdgePlan, x_full, W, attn_l, attn_r, bias):
    """x_full: [n_nodes, n_in] fp32. Returns list of per-core input dicts."""
    n_in, n_h, heads, hid = cfg.n_in, cfg.n_h, cfg.heads, cfg.hid
    wcols = cfg.wcols
    # fold attention vectors: Wl = W @ blockdiag(attn_l)
    Bl = np.zeros((n_h, heads), np.float32)
    Br = np.zeros((n_h, heads), np.float32)
    for h in range(heads):
        Bl[h * hid:(h + 1) * hid, h] = attn_l[h]
        Br[h * hid:(h + 1) * hid, h] = attn_r[h]
    Wc = np.concatenate([W, W @ Bl, W @ Br], axis=1).astype(np.float32)  # [n_in, wcols]
    kchunks = n_in // P
    Wcat_host = Wc.reshape(kchunks, P, wcols).transpose(1, 0, 2).reshape(P, kchunks * wcols)
    Wcat_host = np.ascontiguousarray(Wcat_host).astype(BF16_NP)

    xT = np.zeros((n_in, cfg.n_nodes_pad), BF16_NP)
    xT[:, 0:cfg.n_nodes] = np.ascontiguousarray(x_full.T)

    bias_r = np.ascontiguousarray(np.tile(bias.reshape(1, n_h), (P, 1)).astype(np.float32))
    iota_row = np.ascontiguousarray(
        np.tile(np.arange(P, dtype=np.float32).reshape(1, P), (P, 1))).astype(BF16_NP)
    iota_col = np.ascontiguousarray(np.arange(P, dtype=np.float32).reshape(P, 1))
    ones_row = np.ones((1, P), BF16_NP)

    ins = []
    npc = cfg.nodes_per_core
    for c in range(cfg.n_cores):
        xo = np.zeros((n_in, cfg.npad), BF16_NP)
        lo = c * npc
        hi = min((c + 1) * npc, cfg.n_nodes)
        xo[:, 0:hi - lo] = x_full[lo:hi].T
        ins.append({
            "xT": xT,
            "xT_own": np.ascontiguousarray(xo),
            "Wcat": Wcat_host,
            "bias_rep": bias_r,
            "iota_row": iota_row,
            "iota_col": iota_col,
            "ones_row": ones_row,
            "idx_lo": np.ascontiguousarray(plan.idx_lo_w[c]),
            "idx_hi": np.ascontiguousarray(plan.idx_hi_w[c]),
            "dstc": np.ascontiguousarray(plan.dstc_cols[c]).astype(BF16_NP),
            "dstT2": np.ascontiguousarray(plan.dstT2[c]).astype(BF16_NP),
        })
    return ins


def run_gat(emb, src, dst, W1, attn_l1, attn_r1, bias1, W2, attn_l2, attn_r2, bias2,
            n_nodes=N_NODES, split=32768, trace=False, tmpdir=None):
    emb = np.asarray(emb, np.float32)
    n_in = emb.shape[1]
    cfg1 = LayerCfg(n_in, HEADS * HID, HEADS, 320, n_nodes, N_CORES, split, elu=True)
    cfg2 = LayerCfg(HEADS * HID, OUT_DIM, 1, 64, n_nodes, N_CORES, split, elu=False)
    plan = EdgePlan(src, dst, cfg1)
    # annotate abs base col per gather group (for SEL8 build)
    plan.abs_base = {}
    pos = 0
    for gi, ni in enumerate(plan.glo_sizes):
        plan.abs_base[("lo", gi)] = pos
        pos += ni // P
    for gi, ni in enumerate(plan.ghi_sizes):
        plan.abs_base[("hi", gi)] = pos
        pos += ni // P

    nc1 = build_layer(cfg1, plan)
    ins1 = _prep_layer_inputs(cfg1, plan, emb, np.asarray(W1, np.float32),
                              np.asarray(attn_l1, np.float32),
                              np.asarray(attn_r1, np.float32),
                              np.asarray(bias1, np.float32))
    res1 = run_bass_kernel_spmd(nc1, ins1, list(range(N_CORES)), trace=trace,
                                tmpdir=None if tmpdir is None else tmpdir + "_l1")
    npc = cfg1.nodes_per_core
    x2 = np.concatenate(
        [res1.results[c]["out_x"][0:min(npc, n_nodes - c * npc)] for c in range(N_CORES)],
        axis=0)  # [n_nodes, 256]

    nc2 = build_layer(cfg2, plan)
    ins2 = _prep_layer_inputs(cfg2, plan, x2, np.asarray(W2, np.float32),
                              np.asarray(attn_l2, np.float32).reshape(1, OUT_DIM),
                              np.asarray(attn_r2, np.float32).reshape(1, OUT_DIM),
                              np.asarray(bias2, np.float32))
    res2 = run_bass_kernel_spmd(nc2, ins2, list(range(N_CORES)), trace=trace,
                                tmpdir=None if tmpdir is None else tmpdir + "_l2")
    out = np.concatenate(
        [res2.results[c]["out_x"][0:min(npc, n_nodes - c * npc)] for c in range(N_CORES)],
        axis=0)
    exec_ns = [res1.exec_time_ns, res2.exec_time_ns]
    return out.astype(np.float32), exec_ns


def kernel(emb, src, dst, W1, attn_l1, attn_r1, bias1, W2, attn_l2, attn_r2, bias2):
    out, _ = run_gat(emb, src, dst, W1, attn_l1, attn_r1, bias1,
                     W2, attn_l2, attn_r2, bias2)
    return out


# revision 18
# speedup vs baseline: 1.5714x; 1.0384x over previous
"""Trainium2 Bass kernel for a 2-layer GAT node classifier (SPMD over 8 NeuronCores).

Strategy (per layer):
  - Replicated dense phase: every core computes the full projection table
    H'[n] = [x @ W | x @ (W B_l)] (h plus the per-head left-attention dot),
    written to per-core HBM gather tables. The right-attention dot er is kept
    only for the core's own destination-node range, resident in SBUF.
  - Edge phase: destination nodes are sharded contiguously across cores
    (6250 per core). Per core, edges sorted by destination, grouped into
    128-node destination blocks, padded to 128-edge tiles, and split into
    "lo"/"hi" source groups so the int16 gather indices can address the
    whole 50048-row table via two base tables.
  - Per 1024-edge gather group (one SWDGE dma_gather instruction): build
    selection matrices SEL (edges x nodes one-hot by destination) and its
    transpose via iota-compare + a K=1 PE broadcast matmul, compute
    unnormalized attention weights w = exp(leaky_relu(el[src] + er[dst])),
    and accumulate [sum w*h[src] | sum w] per destination block with PE
    matmuls (exact segment-sum via one-hot matmul). The edge softmax
    denominator is applied per node at the end (alpha never materialized;
    segment-max is skipped — scores are O(1) so exp is safe).
  - Epilogue per block: divide by denominator, add bias, (layer 1: ELU),
    write the core's output slice.

The host only does index preprocessing (graph partition / sort / padding),
weight repacking (folding attention vectors into the weight matrix:
W @ blockdiag(attn)), transposes of inputs, and concatenation of outputs.
All floating-point compute on the 800k edges / 50k nodes runs on device.
"""

import math
import numpy as np
import ml_dtypes

BF16_NP = ml_dtypes.bfloat16

import concourse.bacc as bacc
import concourse.tile as tile
from concourse.tile_rust import add_dep_helper
from concourse import mybir
from concourse.bass_utils import run_bass_kernel_spmd

P = 128
N_CORES = 8
AluOp = mybir.AluOpType
ActFn = mybir.ActivationFunctionType
F32 = mybir.dt.float32
BF16 = mybir.dt.bfloat16
I16 = mybir.dt.int16

# Problem constants (nn_GAT_Node_Classifier)
N_NODES = 50000
N_EDGES = 800000
IN_DIM = 256
HID = 32
HEADS = 8
OUT_DIM = 16
NEG_SLOPE = 0.2


class LayerCfg:
    def __init__(self, n_in, n_h, heads, elem, n_nodes, n_cores, split, elu):
        self.n_in = n_in              # input feature dim (must be mult of 128)
        self.n_h = n_h                # heads * hid
        self.heads = heads
        self.hid = n_h // heads
        self.elem = elem              # gather row floats (>= n_h + heads, 64B mult)
        self.n_cores = n_cores
        self.nodes_per_core = n_nodes // n_cores
        self.blocks = (self.nodes_per_core + P - 1) // P
        self.npad = self.blocks * P
        self.n_nodes = n_nodes
        self.n_nodes_pad = ((n_nodes + P - 1) // P) * P
        self.nt = self.n_nodes_pad // P   # node tiles for the table build
        self.split = min(split, self.n_nodes_pad)
        self.elu = elu
        self.wcols = n_h + 2 * heads
        assert (self.elem * 2) % 256 == 0 and self.elem >= self.n_h + self.heads
        assert self.split % P == 0


class EdgePlan:
    """Host-side edge structures, uniform across cores (SPMD)."""

    def __init__(self, src, dst, cfg: LayerCfg):
        nc_, npc, blocks, split = cfg.n_cores, cfg.nodes_per_core, cfg.blocks, cfg.split
        src = np.asarray(src, dtype=np.int64)
        dst = np.asarray(dst, dtype=np.int64)
        core = dst // npc
        dstl = dst - core * npc
        blk = dstl // P
        dstb = (dstl - blk * P).astype(np.float32)
        grp = (src >= split).astype(np.int64)
        key = (core * blocks + blk) * 2 + grp
        order = np.argsort(key, kind="stable")
        cnt = np.bincount(key, minlength=nc_ * blocks * 2).reshape(nc_, blocks, 2)
        T = -(-cnt.max(axis=0) // P)          # [blocks, 2] tiles per block/group
        T[:, 0] = np.maximum(T[:, 0], 1)      # guarantee psum init per block
        self.T = T
        self.TL = int(T[:, 0].sum())
        self.TH = int(T[:, 1].sum())
        self.T_ALL = self.TL + self.TH
        self.lo_start = np.concatenate([[0], np.cumsum(T[:, 0])])[:-1]
        self.hi_start = np.concatenate([[0], np.cumsum(T[:, 1])])[:-1]
        ptr = np.concatenate([[0], np.cumsum(cnt.reshape(-1))])

        idx_lo = np.zeros((nc_, self.TL * P), np.int16)
        idx_hi = np.zeros((nc_, max(self.TH, 1) * P), np.int16)
        dstc = np.full((nc_, self.T_ALL * P), -1.0, np.float32)
        for c in range(nc_):
            for b in range(blocks):
                for g in range(2):
                    k = (c * blocks + b) * 2 + g
                    e0, e1 = int(ptr[k]), int(ptr[k + 1])
                    if e1 == e0:
                        continue
                    eidx = order[e0:e1]
                    n = e1 - e0
                    if g == 0:
                        base = int(self.lo_start[b]) * P
                        idx_lo[c, base:base + n] = (src[eidx]).astype(np.int16)
                        sbase = base
                    else:
                        base = int(self.hi_start[b]) * P
                        idx_hi[c, base:base + n] = (src[eidx] - split).astype(np.int16)
                        sbase = self.TL * P + base
                    dstc[c, sbase:sbase + n] = dstb[eidx]

        # gather instruction split (1024 idx each, tail partial)
        def gsizes(n_tiles):
            out = []
            rem = n_tiles
            while rem > 0:
                t = min(8, rem)
                out.append(t * P)
                rem -= t
            return out

        self.glo_sizes = gsizes(self.TL)
        self.ghi_sizes = gsizes(self.TH)

        def wrap_idx(flat_all, sizes):
            ng = max(len(sizes), 1)
            out = np.zeros((nc_, P, ng * 64), np.int16)
            for c in range(nc_):
                pos = 0
                for gi, ni in enumerate(sizes):
                    blk16 = flat_all[c, pos:pos + ni].reshape(ni // 16, 16).T
                    out[c, :, gi * 64: gi * 64 + ni // 16] = np.tile(blk16, (8, 1))
                    pos += ni
            return out

        self.idx_lo_w = wrap_idx(idx_lo, self.glo_sizes)
        self.idx_hi_w = wrap_idx(idx_hi, self.ghi_sizes)

        # dstc columns [128, T_ALL]: column t = dst-in-block of the tile's edges
        self.dstc_cols = dstc.reshape(nc_, self.T_ALL, P).transpose(0, 2, 1).copy()

        # dstT2: one row per gather (lo gathers then hi gathers): the gather's
        # edge destinations laid out along the free dim, for the K=1 PE
        # broadcast matmul.
        self.n_gathers = len(self.glo_sizes) + len(self.ghi_sizes)
        self.dstT2 = np.full((nc_, max(self.n_gathers, 1), 1024), -1.0, np.float32)
        gq = 0
        pos = 0
        for ni in self.glo_sizes + self.ghi_sizes:
            self.dstT2[:, gq, 0:ni] = dstc[:, pos:pos + ni]
            pos += ni
            gq += 1

        # per-block tile lists: (stream, stream_tile_idx, abs_t)
        self.block_tiles = []
        for b in range(blocks):
            tl = [("lo", int(self.lo_start[b]) + i, int(self.lo_start[b]) + i)
                  for i in range(int(T[b, 0]))]
            th = [("hi", int(self.hi_start[b]) + i, self.TL + int(self.hi_start[b]) + i)
                  for i in range(int(T[b, 1]))]
            self.block_tiles.append(tl + th)
        # map stream tile -> block
        self.tile_block = {}
        for b, tl in enumerate(self.block_tiles):
            for (s, t, a) in tl:
                self.tile_block[(s, t)] = b


def build_layer(cfg: LayerCfg, plan: EdgePlan):
    nc = bacc.Bacc("TRN2", target_bir_lowering=False, debug=False,
                   num_devices=cfg.n_cores, dynamic_dma_scratch_size=65536,
                   num_swdge_queues=4)
    n_in, n_h, heads, hid = cfg.n_in, cfg.n_h, cfg.heads, cfg.hid
    elem, wcols = cfg.elem, cfg.wcols
    kchunks = n_in // P
    NL = cfg.split
    NH = cfg.n_nodes_pad - cfg.split

    xT = nc.dram_tensor("xT", [n_in, cfg.n_nodes_pad], BF16, kind="ExternalInput").ap()
    xT_own = nc.dram_tensor("xT_own", [n_in, cfg.npad], BF16, kind="ExternalInput").ap()
    Wcat = nc.dram_tensor("Wcat", [P, kchunks * wcols], BF16, kind="ExternalInput").ap()
    bias_rep = nc.dram_tensor("bias_rep", [P, n_h], F32, kind="ExternalInput").ap()
    iota_row = nc.dram_tensor("iota_row", [P, P], BF16, kind="ExternalInput").ap()
    iota_col = nc.dram_tensor("iota_col", [P, 1], F32, kind="ExternalInput").ap()
    ones_row = nc.dram_tensor("ones_row", [1, P], BF16, kind="ExternalInput").ap()
    nglo = max(len(plan.glo_sizes), 1)
    nghi = max(len(plan.ghi_sizes), 1)
    idx_lo = nc.dram_tensor("idx_lo", [P, nglo * 64], I16, kind="ExternalInput").ap()
    idx_hi = nc.dram_tensor("idx_hi", [P, nghi * 64], I16, kind="ExternalInput").ap()
    dstc = nc.dram_tensor("dstc", [P, plan.T_ALL], BF16, kind="ExternalInput").ap()
    dstT2 = nc.dram_tensor("dstT2", [max(plan.n_gathers, 1), 1024], BF16, kind="ExternalInput").ap()
    out_x = nc.dram_tensor("out_x", [cfg.npad, n_h], F32, kind="ExternalOutput").ap()
    dbg = getattr(cfg, "debug", False)
    if dbg:
        dbg_pout = nc.dram_tensor("dbg_pout", [cfg.npad, n_h + heads], F32,
                                  kind="ExternalOutput").ap()
        dbg_gbuf = nc.dram_tensor("dbg_gbuf", [8, P, 8 * elem], F32,
                                  kind="ExternalOutput").ap()
        dbg_wg = nc.dram_tensor("dbg_wg", [8, P, 8 * (n_h + heads)], F32,
                                kind="ExternalOutput").ap()
        dbg_w8 = nc.dram_tensor("dbg_w8", [8, P, 8 * heads], F32,
                                kind="ExternalOutput").ap()
    tab_lo = nc.dram_tensor("tab_lo", [NL, elem], BF16).ap()
    tab_hi = nc.dram_tensor("tab_hi", [max(NH, P), elem], BF16).ap()

    with tile.TileContext(nc) as tc:
        with tc.tile_pool(name="const", bufs=1) as cpool:
            Wcat_sb = cpool.tile([P, kchunks * wcols], BF16)
            nc.sync.dma_start(out=Wcat_sb[:], in_=Wcat[:])
            bias_sb = cpool.tile([P, n_h], F32)
            nc.sync.dma_start(out=bias_sb[:], in_=bias_rep[:])
            ir_sb = cpool.tile([P, P], BF16)
            nc.sync.dma_start(out=ir_sb[:], in_=iota_row[:])
            ic_sb = cpool.tile([P, 1], F32)
            nc.sync.dma_start(out=ic_sb[:], in_=iota_col[:])
            ones_sb = cpool.tile([1, P], BF16)
            nc.sync.dma_start(out=ones_sb[:], in_=ones_row[:])
            ixlo_sb = cpool.tile([P, nglo * 64], I16)
            nc.sync.dma_start(out=ixlo_sb[:], in_=idx_lo[:])
            ixhi_sb = cpool.tile([P, nghi * 64], I16)
            nc.sync.dma_start(out=ixhi_sb[:], in_=idx_hi[:])
            dstc_sb = cpool.tile([P, plan.T_ALL], BF16)
            nc.sync.dma_start(out=dstc_sb[:], in_=dstc[:])
            er_all = cpool.tile([P, cfg.blocks * heads], BF16)

            # ---- Phase A: projection tables ----
            # batched: GT node tiles per DMA group to amortize DMA issue cost
            GT = 4
            tab_writes = []
            with tc.tile_pool(name="pa_sb", bufs=3) as apool, \
                 tc.tile_pool(name="pa_ps", bufs=4, space="PSUM") as appool:
                assert NL % (GT * P) == 0
                for j0 in range(0, cfg.nt, GT):
                    gsz = min(GT, cfg.nt - j0)
                    xa = []
                    for k in range(kchunks):
                        t = apool.tile([P, GT * P], BF16, tag=f"x{k}")
                        nc.sync.dma_start(
                            out=t[:, 0:gsz * P],
                            in_=xT[k * P:(k + 1) * P, j0 * P:(j0 + gsz) * P])
                        xa.append(t)
                    stage = apool.tile([P, GT, n_h + heads], BF16, tag="stage")
                    for jj in range(gsz):
                        ps = appool.tile([P, wcols], F32, tag="ps")
                        for k in range(kchunks):
                            nc.tensor.matmul(
                                out=ps[:], lhsT=xa[k][:, jj * P:(jj + 1) * P],
                                rhs=Wcat_sb[:, k * wcols:k * wcols + wcols],
                                start=(k == 0), stop=(k == kchunks - 1))
                        nc.vector.tensor_copy(stage[:, jj, :], ps[:, 0:n_h + heads])
                    if j0 * P < NL:
                        dst_ap = tab_lo[j0 * P:(j0 + gsz) * P, 0:n_h + heads]
                    else:
                        r0 = j0 * P - NL
                        dst_ap = tab_hi[r0:r0 + gsz * P, 0:n_h + heads]
                    dst_ap = dst_ap.rearrange("(g p) c -> p g c", p=P)
                    tab_writes.append(
                        nc.sync.dma_start(out=dst_ap, in_=stage[:, 0:gsz, :]))
                # er for own nodes -> SBUF resident
                for b0 in range(0, cfg.blocks, GT):
                    gsz = min(GT, cfg.blocks - b0)
                    xa = []
                    for k in range(kchunks):
                        t = apool.tile([P, GT * P], BF16, tag=f"xo{k}")
                        nc.sync.dma_start(
                            out=t[:, 0:gsz * P],
                            in_=xT_own[k * P:(k + 1) * P, b0 * P:(b0 + gsz) * P])
                        xa.append(t)
                    for jj in range(gsz):
                        b = b0 + jj
                        ps = appool.tile([P, heads], F32, tag="pser")
                        for k in range(kchunks):
                            nc.tensor.matmul(
                                out=ps[:], lhsT=xa[k][:, jj * P:(jj + 1) * P],
                                rhs=Wcat_sb[:, k * wcols + n_h + heads:k * wcols + n_h + 2 * heads],
                                start=(k == 0), stop=(k == kchunks - 1))
                        nc.vector.tensor_copy(er_all[:, b * heads:(b + 1) * heads], ps[:])

            # ---- Phase B: edge processing ----
            # fence: every gather must run after all phase-A table writes
            fence_tile = cpool.tile([1, 1], F32)
            fence = nc.vector.memset(fence_tile[:], 0.0)
            for wi in tab_writes:
                add_dep_helper(fence.ins, wi.ins, True, "gather tables written")
            with tc.tile_pool(name="glo", bufs=3) as glo_pool, \
                 tc.tile_pool(name="ghi", bufs=3) as ghi_pool, \
                 tc.tile_pool(name="sel", bufs=4) as sel_pool, \
                 tc.tile_pool(name="wg", bufs=4) as wg_pool, \
                 tc.tile_pool(name="sw", bufs=4) as sw_pool, \
                 tc.tile_pool(name="ep", bufs=2) as ep_pool, \
                 tc.tile_pool(name="ps_bc", bufs=2, space="PSUM") as bc_pool, \
                 tc.tile_pool(name="ps_er", bufs=2, space="PSUM") as er_pool, \
                 tc.tile_pool(name="ps_out", bufs=2, space="PSUM") as out_pool:

                group_data = {}

                def ensure_gather(strm, gi):
                    if (strm, gi) in group_data:
                        return group_data[(strm, gi)]
                    if strm == "lo":
                        ni = plan.glo_sizes[gi]
                        gq = gi
                        pool_, tab, ixsb = glo_pool, tab_lo, ixlo_sb
                    else:
                        ni = plan.ghi_sizes[gi]
                        gq = len(plan.glo_sizes) + gi
                        pool_, tab, ixsb = ghi_pool, tab_hi, ixhi_sb
                    ngt = ni // P
                    buf = pool_.tile([P, 8, elem], BF16, tag="g" + strm)
                    gins = nc.gpsimd.dma_gather(
                        buf[:, 0:ngt, :], tab[:],
                        ixsb[:, gi * 64:gi * 64 + ni // 16],
                        ni, ni, elem, queue_num=gq % 4)
                    add_dep_helper(gins.ins, fence.ins, True, "gather after fence")
                    if dbg and gq < 8:
                        nc.sync.dma_start(out=dbg_gbuf[gq, :, :],
                                          in_=buf[:].rearrange("p a b -> p (a b)"))
                    # broadcast dst rows for this gather: psum_bc[n, e] = dst[e]
                    dr = sw_pool.tile([1, 1024], BF16, tag="dr")
                    nc.sync.dma_start(out=dr[:, 0:ni], in_=dstT2[gq:gq + 1, 0:ni])
                    bc = bc_pool.tile([P, 1024], F32, tag="bc")
                    for h in range(0, ni, 512):
                        w = min(512, ni - h)
                        nc.tensor.matmul(
                            out=bc[:, h:h + w], lhsT=ones_sb[:],
                            rhs=dr[:, h:h + w],
                            start=True, stop=True)
                    selt8 = sel_pool.tile([P, 1024], BF16, tag="selt")
                    nc.vector.tensor_scalar(
                        selt8[:, 0:ni], bc[:, 0:ni], ic_sb[:], None, AluOp.is_equal)
                    # SEL8: one-hot along free (node) axis per tile
                    sel8 = sel_pool.tile([P, 1024], BF16, tag="sel8")
                    t0_abs = plan.abs_base[(strm, gi)]
                    in1 = dstc_sb[:, t0_abs:t0_abs + ngt].unsqueeze(2).to_broadcast([P, ngt, P])
                    nc.vector.tensor_tensor(
                        out=sel8[:, 0:ni].rearrange("p (t n) -> p t n", t=ngt),
                        in0=ir_sb[:].unsqueeze(1).to_broadcast([P, ngt, P]),
                        in1=in1, op=AluOp.is_equal)
                    # er per edge: er8[:, r*heads:...] = selt8_r^T-matmul er_blk
                    er8 = er_pool.tile([P, 8 * heads], F32, tag="er8")
                    for r in range(ngt):
                        bb = plan.tile_block[(strm, gi * 8 + r)]
                        nc.tensor.matmul(
                            out=er8[:, r * heads:(r + 1) * heads],
                            lhsT=selt8[:, r * P:(r + 1) * P],
                            rhs=er_all[:, bb * heads:(bb + 1) * heads],
                            start=True, stop=True)
                    # s = el + er ; w = exp(leaky_relu(s))
                    s8 = sw_pool.tile([P, 8 * heads], F32, tag="s8")
                    nc.vector.tensor_tensor(
                        out=s8[:, 0:ngt * heads].rearrange("p (t h) -> p t h", t=ngt),
                        in0=er8[:, 0:ngt * heads].rearrange("p (t h) -> p t h", t=ngt),
                        in1=buf[:, 0:ngt, n_h:n_h + heads], op=AluOp.add)
                    lr8 = sw_pool.tile([P, 8 * heads], F32, tag="lr8")
                    nc.vector.scalar_tensor_tensor(
                        lr8[:, 0:ngt * heads], s8[:, 0:ngt * heads], NEG_SLOPE,
                        s8[:, 0:ngt * heads], AluOp.mult, AluOp.max)
                    w8 = sw_pool.tile([P, 8 * heads], BF16, tag="w8")
                    nc.scalar.activation(w8[:, 0:ngt * heads], lr8[:, 0:ngt * heads],
                                         ActFn.Exp)
                    # WG8 = [h * w | w] per tile (single 264-wide matmul rhs)
                    nf = n_h + heads
                    wg8 = wg_pool.tile([P, 8 * nf], BF16, tag="wg8")
                    wg8v = wg8[:].rearrange("p (t f) -> p t f", f=nf)
                    nc.vector.tensor_tensor(
                        out=wg8v[:, 0:ngt, 0:n_h].rearrange(
                            "p t (h d) -> p t h d", h=heads),
                        in0=buf[:, 0:ngt, 0:n_h].rearrange(
                            "p t (h d) -> p t h d", h=heads),
                        in1=w8[:, 0:ngt * heads].rearrange(
                            "p (t h) -> p t h", t=ngt).unsqueeze(3).to_broadcast(
                            [P, ngt, heads, hid]),
                        op=AluOp.mult)
                    nc.vector.tensor_copy(
                        wg8v[:, 0:ngt, n_h:nf],
                        w8[:, 0:ngt * heads].rearrange("p (t h) -> p t h", t=ngt))
                    if dbg and gq < 8:
                        nc.sync.dma_start(out=dbg_wg[gq, :, :], in_=wg8[:])
                        nc.sync.dma_start(out=dbg_w8[gq, :, :], in_=w8[:])
                    group_data[(strm, gi)] = (sel8, wg8, w8)
                    return group_data[(strm, gi)]

                for b in range(cfg.blocks):
                    tiles = plan.block_tiles[b]
                    pout = out_pool.tile([P, n_h + heads], F32, tag="pout")
                    for i, (strm, t, abs_t) in enumerate(tiles):
                        gi, r = divmod(t, 8)
                        sel8, wg8, w8 = ensure_gather(strm, gi)
                        st = (i == 0)
                        sp = (i == len(tiles) - 1)
                        nf = n_h + heads
                        nc.tensor.matmul(
                            out=pout[:], lhsT=sel8[:, r * P:(r + 1) * P],
                            rhs=wg8[:, r * nf:(r + 1) * nf], start=st, stop=sp)
                    # epilogue
                    if dbg:
                        dtile = ep_pool.tile([P, n_h + heads], F32, tag="dbg")
                        nc.scalar.copy(dtile[:], pout[:])
                        nc.sync.dma_start(out=dbg_pout[b * P:(b + 1) * P, :], in_=dtile[:])
                    dn = ep_pool.tile([P, heads], F32, tag="dn")
                    nc.vector.tensor_scalar_add(dn[:], pout[:, n_h:n_h + heads], 1e-30)
                    rec = ep_pool.tile([P, heads], F32, tag="rec")
                    nc.vector.reciprocal(rec[:], dn[:])
                    ox = ep_pool.tile([P, n_h], F32, tag="ox")
                    nc.vector.tensor_tensor(
                        out=ox[:].rearrange("p (h d) -> p h d", h=heads),
                        in0=pout[:, 0:n_h].rearrange("p (h d) -> p h d", h=heads),
                        in1=rec[:].unsqueeze(2).to_broadcast([P, heads, hid]),
                        op=AluOp.mult)
                    nc.vector.tensor_add(ox[:], ox[:], bias_sb[:])
                    if cfg.elu:
                        mn = ep_pool.tile([P, n_h], F32, tag="mn")
                        nc.vector.tensor_scalar_min(mn[:], ox[:], 0.0)
                        em = ep_pool.tile([P, n_h], F32, tag="em")
                        nc.scalar.activation(em[:], mn[:], ActFn.Exp)
                        mx = ep_pool.tile([P, n_h], F32, tag="mx")
                        nc.vector.tensor_scalar_max(mx[:], ox[:], 0.0)
                        nc.vector.tensor_add(ox[:], mx[:], em[:])
                        nc.vector.tensor_scalar_add(ox[:], ox[:], -1.0)
                    nc.sync.dma_start(out=out_x[b * P:(b + 1) * P, :], in_=ox[:])

    nc.compile()
    return nc


def _prep_layer_inputs(cfg: LayerCfg, plan: EdgePlan, x_full, W, attn_l, attn_r, bias):
    """x_full: [n_nodes, n_in] fp32. Returns list of per-core input dicts."""
    n_in, n_h, heads, hid = cfg.n_in, cfg.n_h, cfg.heads, cfg.hid
    wcols = cfg.wcols
    # fold attention vectors: Wl = W @ blockdiag(attn_l)
    Bl = np.zeros((n_h, heads), np.float32)
    Br = np.zeros((n_h, heads), np.float32)
    for h in range(heads):
        Bl[h * hid:(h + 1) * hid, h] = attn_l[h]
        Br[h * hid:(h + 1) * hid, h] = attn_r[h]
    Wc = np.concatenate([W, W @ Bl, W @ Br], axis=1).astype(np.float32)  # [n_in, wcols]
    kchunks = n_in // P
    Wcat_host = Wc.reshape(kchunks, P, wcols).transpose(1, 0, 2).reshape(P, kchunks * wcols)
    Wcat_host = np.ascontiguousarray(Wcat_host).astype(BF16_NP)

    xT = np.zeros((n_in, cfg.n_nodes_pad), BF16_NP)
    xT[:, 0:cfg.n_nodes] = np.ascontiguousarray(x_full.T)

    bias_r = np.ascontiguousarray(np.tile(bias.reshape(1, n_h), (P, 1)).astype(np.float32))
    iota_row = np.ascontiguousarray(
        np.tile(np.arange(P, dtype=np.float32).reshape(1, P), (P, 1))).astype(BF16_NP)
    iota_col = np.ascontiguousarray(np.arange(P, dtype=np.float32).reshape(P, 1))
    ones_row = np.ones((1, P), BF16_NP)

    ins = []
    npc = cfg.nodes_per_core
    for c in range(cfg.n_cores):
        xo = np.zeros((n_in, cfg.npad), BF16_NP)
        lo = c * npc
        hi = min((c + 1) * npc, cfg.n_nodes)
        xo[:, 0:hi - lo] = x_full[lo:hi].T
        ins.append({
            "xT": xT,
            "xT_own": np.ascontiguousarray(xo),
            "Wcat": Wcat_host,
            "bias_rep": bias_r,
            "iota_row": iota_row,
            "iota_col": iota_col,
            "ones_row": ones_row,
            "idx_lo": np.ascontiguousarray(plan.idx_lo_w[c]),
            "idx_hi": np.ascontiguousarray(plan.idx_hi_w[c]),
            "dstc": np.ascontiguousarray(plan.dstc_cols[c]).astype(BF16_NP),
            "dstT2": np.ascontiguousarray(plan.dstT2[c]).astype(BF16_NP),
        })
    return ins


def run_gat(emb, src, dst, W1, attn_l1, attn_r1, bias1, W2, attn_l2, attn_r2, bias2,
            n_nodes=N_NODES, split=32768, trace=False, tmpdir=None):
    emb = np.asarray(emb, np.float32)
    n_in = emb.shape[1]
    cfg1 = LayerCfg(n_in, HEADS * HID, HEADS, 384, n_nodes, N_CORES, split, elu=True)
    cfg2 = LayerCfg(HEADS * HID, OUT_DIM, 1, 128, n_nodes, N_CORES, split, elu=False)
    plan = EdgePlan(src, dst, cfg1)
    # annotate abs base col per gather group (for SEL8 build)
    plan.abs_base = {}
    pos = 0
    for gi, ni in enumerate(plan.glo_sizes):
        plan.abs_base[("lo", gi)] = pos
        pos += ni // P
    for gi, ni in enumerate(plan.ghi_sizes):
        plan.abs_base[("hi", gi)] = pos
        pos += ni // P

    nc1 = build_layer(cfg1, plan)
    ins1 = _prep_layer_inputs(cfg1, plan, emb, np.asarray(W1, np.float32),
                              np.asarray(attn_l1, np.float32),
                              np.asarray(attn_r1, np.float32),
                              np.asarray(bias1, np.float32))
    res1 = run_bass_kernel_spmd(nc1, ins1, list(range(N_CORES)), trace=trace,
                                tmpdir=None if tmpdir is None else tmpdir + "_l1")
    npc = cfg1.nodes_per_core
    x2 = np.concatenate(
        [res1.results[c]["out_x"][0:min(npc, n_nodes - c * npc)] for c in range(N_CORES)],
        axis=0)  # [n_nodes, 256]

    nc2 = build_layer(cfg2, plan)
    ins2 = _prep_layer_inputs(cfg2, plan, x2, np.asarray(W2, np.float32),
                              np.asarray(attn_l2, np.float32).reshape(1, OUT_DIM),
                              np.asarray(attn_r2, np.float32).reshape(1, OUT_DIM),
                              np.asarray(bias2, np.float32))
    res2 = run_bass_kernel_spmd(nc2, ins2, list(range(N_CORES)), trace=trace,
                                tmpdir=None if tmpdir is None else tmpdir + "_l2")
    out = np.concatenate(
        [res2.results[c]["out_x"][0:min(npc, n_nodes - c * npc)] for c in range(N_CORES)],
        axis=0)
    exec_ns = [res1.exec_time_ns, res2.exec_time_ns]
    return out.astype(np.float32), exec_ns


def kernel(emb, src, dst, W1, attn_l1, attn_r1, bias1, W2, attn_l2, attn_r2, bias2):
    out, _ = run_gat(emb, src, dst, W1, attn_l1, attn_r1, bias1,
                     W2, attn_l2, attn_r2, bias2)
    return out
